# revision 38
# baseline (speedup 1.0000x reference)
"""Trainium2 Bass kernel for nn_Decoder: 2-layer GRU decoder + LayerNorm + ELU + vocab head.

Contract: kernel(**inputs) takes the FULL unsharded inputs and returns the FULL
(512, 64, 10000) float32 logits. Data-parallel: batch 512 -> 8 cores x 64.

v2 design (all-transposed, tanh-only, fp16):
- State kept TRANSPOSED the whole time: Shist [128 part = h-component-in-chunk,
  131 blocks, 2 h-chunks, 64 batch] fp16. Block q: q=0 init state; q=2s+1 =
  L0 state step s; q=2s+2 = L1 state (= output y1) step s. Slot s computes
  L0 step s and L1 step s-1 fused; their h' outputs land in ADJACENT blocks
  [2s, 2s+1] -> single fused DVE write. No PE transposes anywhere.
- Gate matmuls are weight-stationary ([128,128] fp16 lhsT chunks, FWL hides
  LDWEIGHTS), moving operand = state block [128, 64]. Biases/constants folded
  in as K=1 matmuls.
- sigmoid eliminated: r,z use t=tanh(x/2); r*hn = (1+tr)*hn' with the 0.5
  folded into the Whh n-rows on host; u=(1+tz)/2, v=(1-tz)/2 as cheap DVE
  tensor_scalar ops. ACT therefore only needs Tanh/Exp/Square -> single
  table set (exp_and_others), so ELU = exp(min(y,0))-1+max(y,0) directly.
- LayerNorm in transposed layout: mean/E[h2] via ones-matmul partition
  reductions, rsqrt via bit-trick+Newton on GpSimd, broadcast back via K=1
  ones matmuls; apply+ELU fused over G=4 steps.
- Head: yT (post-LN/ELU, fp16) stationary [128,128] per (pair, chunk); wout
  fp16 moving N=500; PSUM [128, 2, 512] (pad 12); copies to fp16 staging split
  ACT/DVE; one 2.5MB DMA per timestep-pair, alternating sync/gpsimd queues.
- Output written fp16 (halves DMA), host casts back to f32.
"""
import os
import sys

for _p in ("/opt/trn_rl_repo", "/root/.axon_site/_ro/trn_rl_repo"):
    if os.path.isdir(_p) and _p not in sys.path:
        sys.path.append(_p)

import numpy as np
import concourse.bacc as bacc
import concourse.mybir as mybir
import concourse.tile as tile
from concourse.bass_utils import run_bass_kernel_spmd

F32 = mybir.dt.float32
F16 = mybir.dt.float16
I32 = mybir.dt.int32
AF = mybir.ActivationFunctionType
ALU = mybir.AluOpType

B, Z, H, T, P = 512, 64, 256, 64, 10000
NCORES = 8
BS = B // NCORES
LN_EPS = 1e-5
G = 4                  # LN/head group size (steps)
NCH = 500              # head N-chunk; 2 chunks (1024 cols w/ pad) per PSUM tile
RSQRT_NEWTON = 1

last_exec_ns = None
last_results = None


def _np(x):
    return np.ascontiguousarray(np.asarray(x, dtype=np.float32))


def _f16(x):
    return np.ascontiguousarray(np.asarray(x, dtype=np.float16))


def _build(flags):
    nc = bacc.Bacc("TRN2", target_bir_lowering=False)

    zT_d = nc.dram_tensor("zT", (Z, BS), F16, kind="ExternalInput")
    winitT_d = nc.dram_tensor("winitT", (Z, 2, 128), F16, kind="ExternalInput")
    whh0_d = nc.dram_tensor("whh0", (128, 2, 6, 128), F16, kind="ExternalInput")
    whh1_d = nc.dram_tensor("whh1", (128, 2, 6, 128), F16, kind="ExternalInput")
    wih1_d = nc.dram_tensor("wih1", (128, 2, 6, 128), F16, kind="ExternalInput")
    wout_d = nc.dram_tensor("wout", (128, 2, P), F16, kind="ExternalInput")
    c0rz_d = nc.dram_tensor("c0rz", (1, 2 * H), F16, kind="ExternalInput")
    c0n_d = nc.dram_tensor("c0n", (1, H), F16, kind="ExternalInput")
    opt = {}
    for name, shape in (
        ("binit", (1, H)), ("c1rz", (1, 2 * H)), ("bhh0n", (1, H)),
        ("bhh1n", (1, H)), ("bih1n", (1, H)), ("bout", (1, P)),
    ):
        if flags[name]:
            opt[name] = nc.dram_tensor(name, shape, F16, kind="ExternalInput")
    for name in ("lng", "lnb"):
        if flags[name]:
            opt[name] = nc.dram_tensor(name, (128, 2), F32, kind="ExternalInput")

    # (T, BS, P) layout so each timestep-pair staging DMA is one contiguous
    # 2.56MB block (scattered (b,t)-row writes measured 6x slower); the host
    # un-transposes to (BS, T, P).
    out_d = nc.dram_tensor("out", (T, BS, P), F16, kind="ExternalOutput")

    with tile.TileContext(nc) as tc:
        with (
            tc.tile_pool(name="const", bufs=1) as cp,
            tc.tile_pool(name="work", bufs=2) as wp,
            tc.tile_pool(name="psum", bufs=1, space="PSUM") as pp,
        ):
            # ---- constants / weights into SBUF -----------------------------
            zT = cp.tile([Z, BS], F16)
            winitT = cp.tile([Z, 2, 128], F16)
            whh0 = cp.tile([128, 2, 6, 128], F16)
            whh1 = cp.tile([128, 2, 6, 128], F16)
            wih1 = cp.tile([128, 2, 6, 128], F16)
            wout = cp.tile([128, 2, P], F16)
            c0rz = cp.tile([1, 2 * H], F16)
            c0n = cp.tile([1, H], F16)
            nc.sync.dma_start(out=zT, in_=zT_d[:])
            nc.sync.dma_start(out=winitT, in_=winitT_d[:])
            nc.scalar.dma_start(out=whh0, in_=whh0_d[:])
            nc.scalar.dma_start(out=whh1, in_=whh1_d[:])
            nc.scalar.dma_start(out=wih1, in_=wih1_d[:])
            nc.gpsimd.dma_start(out=wout, in_=wout_d[:])
            nc.sync.dma_start(out=c0rz, in_=c0rz_d[:])
            nc.sync.dma_start(out=c0n, in_=c0n_d[:])
            ot = {}
            for name, t_ in opt.items():
                sh = list(t_.shape)
                dt = F32 if name in ("lng", "lnb") else F16
                ot[name] = cp.tile(sh, dt)
                nc.sync.dma_start(out=ot[name], in_=t_[:])

            ones64 = cp.tile([1, BS], F16)
            nc.vector.memset(ones64, 1.0)
            ones1 = cp.tile([1, 128], F16)
            nc.vector.memset(ones1, 1.0)
            onesK = cp.tile([128, 1], F16)
            nc.vector.memset(onesK, 1.0 / H)

            # state history: block q=0 init; q=2s+1 L0 step s; q=2s+2 L1 step s
            Shist = cp.tile([128, 2 * T + 3, 2, BS], F16)

            # ---- init: h0 = elu(z @ W_init.T + b_init), write block 0 ------
            gi = pp.tile([128, 2, 8, BS], F32, tag="gates", bufs=1)
            for c in range(2):
                nc.tensor.matmul(gi[:, 0, c, :], winitT[:, c, :], zT,
                                 start=True, stop=not flags["binit"])
                if flags["binit"]:
                    nc.tensor.matmul(gi[:, 0, c, :],
                                     ot["binit"][:, c * 128:(c + 1) * 128],
                                     ones64, start=False, stop=True)
            h0pre = wp.tile([128, 2, BS], F16, tag="h0pre")
            nc.vector.tensor_scalar(out=h0pre, in0=gi[:, 0, 0:2, :], scalar1=0.0,
                                    scalar2=None, op0=ALU.min, op1=ALU.bypass)
            h0ex = wp.tile([128, 2, BS], F16, tag="h0ex")
            nc.scalar.activation(out=h0ex, in_=h0pre, func=AF.Exp)
            nc.vector.tensor_scalar(out=h0pre, in0=gi[:, 0, 0:2, :], scalar1=0.0,
                                    scalar2=None, op0=ALU.max, op1=ALU.bypass)
            nc.vector.scalar_tensor_tensor(out=Shist[:, 0, :, :], in0=h0ex,
                                           scalar=-1.0, in1=h0pre,
                                           op0=ALU.add, op1=ALU.add)

            c15 = cp.tile([1, G * BS], F32)
            nc.vector.memset(c15, 1.5)
            ceps = cp.tile([1, G * BS], F32)
            nc.vector.memset(ceps, LN_EPS)
            cm05 = cp.tile([1, G * BS], F32)
            nc.vector.memset(cm05, -0.5)

            # ---- helpers ---------------------------------------------------
            grp = {}
            grp_sb = {}

            def emit_stats(g):
                """Stage A: stats + rsqrt chain for L1 steps [G*g, G*g+G)."""
                # y1 step t lives at block 2t+2 -> stride-2 slice
                q0 = 2 * (G * g) + 2
                Hv = Shist[:, q0:q0 + 2 * G:2, :, :]        # [128, G(t), 2(c), 64]
                HvC = Hv.rearrange("p t c b -> p c t b")    # (c, t, b) view
                hvb = wp.tile([128, 2, G, BS], F16, tag="hvb")
                nc.vector.tensor_copy(out=hvb, in_=HvC)
                sq = wp.tile([128, 2, G, BS], F16, tag="sq")
                nc.scalar.activation(out=sq, in_=hvb, func=AF.Square)
                sb = pp.tile([128, 2, G * BS], F32, tag="sb", bufs=1)
                grp_sb[g] = sb
                st = sb[0:1, :, :]
                nc.tensor.matmul(st[:, 0, :], onesK, hvb[:, 0], start=True,
                                 stop=False)
                nc.tensor.matmul(st[:, 0, :], onesK, hvb[:, 1], start=False,
                                 stop=True)
                nc.tensor.matmul(st[:, 1, :], onesK, sq[:, 0], start=True,
                                 stop=False)
                nc.tensor.matmul(st[:, 1, :], onesK, sq[:, 1], start=False,
                                 stop=True)
                mv = wp.tile([1, 2, G * BS], F32, tag="mv")
                nc.vector.tensor_copy(out=mv, in_=st)
                # ve = var + eps; vh = -0.5*ve  (gpsimd; mv[0]=mu, mv[1]=E[h^2])
                ve = wp.tile([1, G * BS], F32, tag="ve")
                vh = wp.tile([1, G * BS], F32, tag="vh")
                nc.gpsimd.tensor_tensor(out=ve, in0=mv[:, 0, :], in1=mv[:, 0, :],
                                        op=ALU.mult)
                nc.gpsimd.tensor_tensor(out=ve, in0=mv[:, 1, :], in1=ve,
                                        op=ALU.subtract)
                nc.gpsimd.tensor_tensor(out=ve, in0=ve, in1=ceps, op=ALU.add)
                nc.gpsimd.tensor_tensor(out=vh, in0=ve, in1=cm05, op=ALU.mult)
                # rsqrt(ve): bit trick + Newton (tensor-tensor only on gpsimd)
                yi = wp.tile([1, G * BS], I32, tag="yi")
                nc.vector.tensor_scalar(out=yi, in0=ve.bitcast(I32), scalar1=1,
                                        scalar2=None, op0=ALU.logical_shift_right,
                                        op1=ALU.bypass)
                nc.vector.tensor_scalar(out=yi, in0=yi, scalar1=-1,
                                        scalar2=0x5F3759DF, op0=ALU.mult,
                                        op1=ALU.add)
                rs = yi.bitcast(F32)
                tn = wp.tile([1, G * BS], F32, tag="tn")
                for _ in range(RSQRT_NEWTON):
                    nc.gpsimd.tensor_tensor(out=tn, in0=rs, in1=rs, op=ALU.mult)
                    nc.gpsimd.tensor_tensor(out=tn, in0=tn, in1=vh, op=ALU.mult)
                    nc.gpsimd.tensor_tensor(out=tn, in0=tn, in1=c15, op=ALU.add)
                    nc.gpsimd.tensor_tensor(out=rs, in0=rs, in1=tn, op=ALU.mult)
                # fp16 (mu, rs) for broadcast matmuls
                m16 = wp.tile([1, 2, G * BS], F16, tag="m16")
                nc.vector.tensor_copy(out=m16[:, 0, :], in_=mv[:, 0, :])
                nc.vector.tensor_copy(out=m16[:, 1, :], in_=rs)
                grp[g] = {"hvb": hvb, "m16": m16}

            def emit_apply(g):
                """Stage B: broadcast + LN apply + ELU -> yb (fp16, (c,t,b))."""
                d = grp[g]
                hvb, m16 = d["hvb"], d["m16"]
                bc = grp_sb.pop(g)
                nc.tensor.matmul(bc[:, 0, :], ones1, m16[:, 0, :],
                                 start=True, stop=True)
                nc.tensor.matmul(bc[:, 1, :], ones1, m16[:, 1, :],
                                 start=True, stop=True)
                # apply LN: y = (h - mu) * rs  (+ lng/lnb if present)
                bmu = bc[:, 0, :].rearrange("p (t b) -> p t b", t=G) \
                    .unsqueeze(1).broadcast_to([128, 2, G, BS])
                brs = bc[:, 1, :].rearrange("p (t b) -> p t b", t=G) \
                    .unsqueeze(1).broadcast_to([128, 2, G, BS])
                yb = wp.tile([128, 2, G, BS], F16, tag="yb")
                nc.vector.tensor_tensor(out=yb, in0=hvb, in1=bmu, op=ALU.subtract)
                nc.vector.tensor_tensor(out=yb, in0=yb, in1=brs, op=ALU.mult)
                d["yb"] = yb
                if flags["lng"]:
                    for c in range(2):
                        nc.vector.tensor_scalar(out=yb[:, c], in0=yb[:, c],
                                                scalar1=ot["lng"][:, c:c + 1],
                                                scalar2=None, op0=ALU.mult,
                                                op1=ALU.bypass)
                if flags["lnb"]:
                    for c in range(2):
                        nc.vector.tensor_scalar(out=yb[:, c], in0=yb[:, c],
                                                scalar1=ot["lnb"][:, c:c + 1],
                                                scalar2=None, op0=ALU.add,
                                                op1=ALU.bypass)
                # ELU: yp = relu(y); y = exp(y - yp) - 1 + yp
                yp = wp.tile([128, 2, G, BS], F16, tag="yp")
                nc.scalar.activation(out=yp, in_=yb, func=AF.Relu)
                mn = wp.tile([128, 2, G, BS], F16, tag="mn")
                nc.vector.scalar_tensor_tensor(out=mn, in0=yp, scalar=-1.0,
                                               in1=yb, op0=ALU.mult, op1=ALU.add)
                ex = wp.tile([128, 2, G, BS], F16, tag="ex")
                nc.scalar.activation(out=ex, in_=mn, func=AF.Exp)
                nc.vector.scalar_tensor_tensor(out=yb, in0=ex, scalar=-1.0,
                                               in1=yp, op0=ALU.add, op1=ALU.add)

            def emit_half(g, j, h):
                """Half of the head work (5 of 10 q-groups) for pair j of
                group g; the DMA is issued with the second half."""
                yb = grp[g]["yb"]
                t0 = G * g + 2 * j
                if h == 0:
                    stg = wp.tile([128, P], F16, tag="stg", bufs=4)
                    grp[g]["stg%d" % j] = stg
                else:
                    stg = grp[g]["stg%d" % j]
                yT0 = yb[:, 0, 2 * j:2 * j + 2, :]
                yT1 = yb[:, 1, 2 * j:2 * j + 2, :]
                nq = P // NCH
                for n in range(h * nq // 2, (h + 1) * nq // 2):
                    hp = pp.tile([128, 512], F32, tag="hp", bufs=5)
                    nc.tensor.matmul(hp[:, 0:NCH], yT0,
                                     wout[:, 0, n * NCH:(n + 1) * NCH],
                                     start=True, stop=False)
                    nc.tensor.matmul(hp[:, 0:NCH], yT1,
                                     wout[:, 1, n * NCH:(n + 1) * NCH],
                                     start=False, stop=not flags["bout"])
                    if flags["bout"]:
                        nc.tensor.matmul(hp[:, 0:NCH], ones1,
                                         ot["bout"][:, n * NCH:(n + 1) * NCH],
                                         start=False, stop=True)
                    dst = stg[:, n * NCH:(n + 1) * NCH]
                    if n % 2 == 0:
                        nc.vector.tensor_copy(out=dst, in_=hp[:, 0:NCH])
                    else:
                        nc.scalar.copy(out=dst, in_=hp[:, 0:NCH])
                if h == 1:
                    eng = (nc.sync, nc.gpsimd)[(2 * g + j) % 2]
                    eng.dma_start(out=out_d[t0:t0 + 2], in_=stg)

            # gate tile slice layout: [128, l(2), kind(8), 64]
            # l: 0=L1, 1=L0.  kind: 0,1=r(c0,c1) 2,3=z 4,5=hn' 6,7=xn
            # ---- main loop -------------------------------------------------
            for s in range(T + 1):
                L0 = s < T
                L1 = s >= 1
                lo = 0 if L1 else 1      # active l-slice range [lo:hi]
                hi = 2 if L0 else 1
                nl = hi - lo

                hp_ctx = tc.high_priority()
                hp_ctx.__enter__()
                gt = pp.tile([128, 2, 8, BS], F32, tag="gates", bufs=1)

                # emission order matters: tanh-r gates only on the r-chunk
                # matmuls, so emit r first, then hn/xn (needed next by th/ta),
                # and z last (needed only after tanh-n).
                rhs_h1 = Shist[:, max(2 * s - 2, 0), :, :]       # y1_{s-2}
                rhs_y0 = Shist[:, max(2 * s - 1, 0), :, :]       # y0_{s-1}
                rhs_h0 = Shist[:, max(2 * s - 1, 0), :, :]       # h0_{s-1}

                def l1_rz(gc):
                    # bias first: it only reads constants, so it can execute
                    # during the previous slot's ladder
                    if flags["c1rz"]:
                        nc.tensor.matmul(gt[:, 0, gc, :],
                                         ot["c1rz"][:, gc * 128:(gc + 1) * 128],
                                         ones64, start=True, stop=False)
                    nc.tensor.matmul(gt[:, 0, gc, :], whh1[:, 0, gc, :],
                                     rhs_h1[:, 0, :], start=not flags["c1rz"],
                                     stop=False)
                    nc.tensor.matmul(gt[:, 0, gc, :], whh1[:, 1, gc, :],
                                     rhs_h1[:, 1, :], start=False, stop=False)
                    nc.tensor.matmul(gt[:, 0, gc, :], wih1[:, 0, gc, :],
                                     rhs_y0[:, 0, :], start=False, stop=False)
                    nc.tensor.matmul(gt[:, 0, gc, :], wih1[:, 1, gc, :],
                                     rhs_y0[:, 1, :], start=False, stop=True)

                def l0_rz(gc):
                    nc.tensor.matmul(gt[:, 1, gc, :],
                                     c0rz[:, gc * 128:(gc + 1) * 128],
                                     ones64, start=True, stop=False)
                    nc.tensor.matmul(gt[:, 1, gc, :], whh0[:, 0, gc, :],
                                     rhs_h0[:, 0, :], start=False, stop=False)
                    nc.tensor.matmul(gt[:, 1, gc, :], whh0[:, 1, gc, :],
                                     rhs_h0[:, 1, :], start=False, stop=True)

                for gc in (0, 1):                                # r chunks
                    if L1:
                        l1_rz(gc)
                    if L0:
                        l0_rz(gc)
                if L1:
                    for ci in range(2):                          # hn', xn
                        nc.tensor.matmul(gt[:, 0, 4 + ci, :], whh1[:, 0, 4 + ci, :],
                                         rhs_h1[:, 0, :], start=True, stop=False)
                        nc.tensor.matmul(gt[:, 0, 4 + ci, :], whh1[:, 1, 4 + ci, :],
                                         rhs_h1[:, 1, :], start=False,
                                         stop=not flags["bhh1n"])
                        if flags["bhh1n"]:
                            nc.tensor.matmul(gt[:, 0, 4 + ci, :],
                                             ot["bhh1n"][:, ci * 128:(ci + 1) * 128],
                                             ones64, start=False, stop=True)
                        nc.tensor.matmul(gt[:, 0, 6 + ci, :], wih1[:, 0, 4 + ci, :],
                                         rhs_y0[:, 0, :], start=True, stop=False)
                        nc.tensor.matmul(gt[:, 0, 6 + ci, :], wih1[:, 1, 4 + ci, :],
                                         rhs_y0[:, 1, :], start=False,
                                         stop=not flags["bih1n"])
                        if flags["bih1n"]:
                            nc.tensor.matmul(gt[:, 0, 6 + ci, :],
                                             ot["bih1n"][:, ci * 128:(ci + 1) * 128],
                                             ones64, start=False, stop=True)
                if L0:
                    for ci in range(2):
                        nc.tensor.matmul(gt[:, 1, 4 + ci, :], whh0[:, 0, 4 + ci, :],
                                         rhs_h0[:, 0, :], start=True, stop=False)
                        nc.tensor.matmul(gt[:, 1, 4 + ci, :], whh0[:, 1, 4 + ci, :],
                                         rhs_h0[:, 1, :], start=False,
                                         stop=not flags["bhh0n"])
                        if flags["bhh0n"]:
                            nc.tensor.matmul(gt[:, 1, 4 + ci, :],
                                             ot["bhh0n"][:, ci * 128:(ci + 1) * 128],
                                             ones64, start=False, stop=True)
                        nc.tensor.matmul(gt[:, 1, 6 + ci, :],
                                         c0n[:, ci * 128:(ci + 1) * 128],
                                         ones64, start=True, stop=True)
                for gc in (2, 3):                                # z chunks (late)
                    if L1:
                        l1_rz(gc)
                    if L0:
                        l0_rz(gc)

                # nonlinearity + blend (fused over active layers)
                # u = (1+tz)/2, v = (1-tz)/2; h' = u*h_prev + v*n
                tr = wp.tile([128, 2, 2, BS], F16, tag="tr")
                tz = wp.tile([128, 2, 2, BS], F16, tag="tz")
                th = wp.tile([128, 2, 2, BS], F16, tag="th")
                ta = wp.tile([128, 2, 2, BS], F16, tag="ta")
                nn = wp.tile([128, 2, 2, BS], F16, tag="nn")
                uu = wp.tile([128, 2, 2, BS], F16, tag="uu")
                vv = wp.tile([128, 2, 2, BS], F16, tag="vv")
                uh = wp.tile([128, 2, 2, BS], F16, tag="uh")
                vn = wp.tile([128, 2, 2, BS], F16, tag="vn")

                A = slice(lo, hi)
                nc.scalar.activation(out=tr[:, A], in_=gt[:, A, 0:2, :],
                                     func=AF.Tanh, scale=0.5)
                nc.vector.scalar_tensor_tensor(out=th[:, A], in0=tr[:, A],
                                               scalar=1.0, in1=gt[:, A, 4:6, :],
                                               op0=ALU.add, op1=ALU.mult)
                nc.vector.tensor_tensor(out=ta[:, A], in0=th[:, A],
                                        in1=gt[:, A, 6:8, :], op=ALU.add)
                nc.scalar.activation(out=tz[:, A], in_=gt[:, A, 2:4, :],
                                     func=AF.Tanh, scale=0.5)
                nc.scalar.activation(out=nn[:, A], in_=ta[:, A], func=AF.Tanh)
                # h_prev blocks: [2s-2]=y1_{s-2}, [2s-1]=h0_{s-1}
                if s == 0:
                    hprev = Shist[:, 0:1, :, :]
                else:
                    base = 2 * s - 2 + lo
                    hprev = Shist[:, base:base + nl, :, :]
                # u/v/uh run on DVE while ACT computes tanh-n
                nc.vector.tensor_scalar(out=uu[:, A], in0=tz[:, A], scalar1=0.5,
                                        scalar2=0.5, op0=ALU.mult, op1=ALU.add)
                nc.vector.tensor_scalar(out=vv[:, A], in0=tz[:, A], scalar1=-0.5,
                                        scalar2=0.5, op0=ALU.mult, op1=ALU.add)
                nc.vector.tensor_tensor(out=uh[:, A], in0=uu[:, A],
                                        in1=hprev, op=ALU.mult)
                nc.vector.tensor_tensor(out=vn[:, A], in0=vv[:, A],
                                        in1=nn[:, A], op=ALU.mult)
                nc.vector.tensor_tensor(
                    out=Shist[:, 2 * s + lo:2 * s + hi, :, :],
                    in0=uh[:, A], in1=vn[:, A], op=ALU.add)
                hp_ctx.__exit__(None, None, None)

                # software-pipelined phase B (one head half-pair per slot):
                #   4g+4: stats(g);  4g+6: apply(g) + pair0.h0;  4g+7: pair0.h1
                #   4g+8: pair1.h0;  4g+9: pair1.h1
                r = s % G
                if r == 0 and s >= G:
                    emit_stats(s // G - 1)
                if r == 2 and s >= 6:
                    emit_apply((s - 6) // G)
                    emit_half((s - 6) // G, 0, 0)
                if r == 3 and s >= 7:
                    emit_half((s - 7) // G, 0, 1)
                if r == 0 and s >= 8:
                    emit_half((s - 8) // G, 1, 0)
                if r == 1 and s >= 9:
                    emit_half((s - 9) // G, 1, 1)

            # tail: finish the pipeline for the last groups
            emit_half(T // G - 2, 1, 1)
            emit_apply(T // G - 1)
            emit_half(T // G - 1, 0, 0)
            emit_half(T // G - 1, 0, 1)
            emit_half(T // G - 1, 1, 0)
            emit_half(T // G - 1, 1, 1)

    nc.compile()
    return nc


_cache = {}


def _get_program(flags):
    key = tuple(sorted(flags.items()))
    if key not in _cache:
        _cache[key] = _build(flags)
    return _cache[key]


def kernel(z, W_init, b_init, embedding, W_ih0, W_hh0, b_ih0, b_hh0,
           W_ih1, W_hh1, b_ih1, b_hh1, ln_g, ln_b, W_out, b_out):
    global last_exec_ns, last_results
    z = _np(z); W_init = _np(W_init); b_init = _np(b_init)
    embedding = _np(embedding)
    W_ih0 = _np(W_ih0); W_hh0 = _np(W_hh0); b_ih0 = _np(b_ih0); b_hh0 = _np(b_hh0)
    W_ih1 = _np(W_ih1); W_hh1 = _np(W_hh1); b_ih1 = _np(b_ih1); b_hh1 = _np(b_hh1)
    ln_g = _np(ln_g); ln_b = _np(ln_b); W_out = _np(W_out); b_out = _np(b_out)

    # layer-0 input gates are constant across (b, t): fold embedding @ W_ih0.T
    gx0 = (embedding @ W_ih0.T + b_ih0).reshape(1, 3 * H)
    c0rz = gx0[:, 0:2 * H] + b_hh0[None, 0:2 * H]
    c0n = gx0[:, 2 * H:]
    c1rz = (b_ih1 + b_hh1)[None, 0:2 * H]

    flags = {
        "binit": bool(np.any(b_init != 0)),
        "c1rz": bool(np.any(c1rz != 0)),
        "bhh0n": bool(np.any(b_hh0[2 * H:] != 0)),
        "bhh1n": bool(np.any(b_hh1[2 * H:] != 0)),
        "bih1n": bool(np.any(b_ih1[2 * H:] != 0)),
        "lng": bool(np.any(ln_g != 1.0)),
        "lnb": bool(np.any(ln_b != 0)),
        "bout": bool(np.any(b_out != 0)),
    }
    nc = _get_program(flags)

    # weight prep: transposed chunked layouts; 0.5 folded into Whh n-rows
    Wt0 = W_hh0.T.copy()
    Wt0[:, 2 * H:] *= 0.5
    Wt1 = W_hh1.T.copy()
    Wt1[:, 2 * H:] *= 0.5
    common = {
        "winitT": _f16(W_init.T.reshape(Z, 2, 128)),
        "whh0": _f16(Wt0.reshape(2, 128, 6, 128).transpose(1, 0, 2, 3)),
        "whh1": _f16(Wt1.reshape(2, 128, 6, 128).transpose(1, 0, 2, 3)),
        "wih1": _f16(W_ih1.T.reshape(2, 128, 6, 128).transpose(1, 0, 2, 3)),
        "wout": _f16(W_out.T.reshape(2, 128, P).transpose(1, 0, 2)),
        "c0rz": _f16(c0rz),
        "c0n": _f16(c0n),
    }
    if flags["binit"]:
        common["binit"] = _f16(b_init.reshape(1, H))
    if flags["c1rz"]:
        common["c1rz"] = _f16(c1rz)
    if flags["bhh0n"]:
        common["bhh0n"] = _f16(0.5 * b_hh0[None, 2 * H:])
    if flags["bhh1n"]:
        common["bhh1n"] = _f16(0.5 * b_hh1[None, 2 * H:])
    if flags["bih1n"]:
        common["bih1n"] = _f16(b_ih1[None, 2 * H:])
    if flags["lng"]:
        common["lng"] = np.ascontiguousarray(ln_g.reshape(2, 128).T)
    if flags["lnb"]:
        common["lnb"] = np.ascontiguousarray(ln_b.reshape(2, 128).T)
    if flags["bout"]:
        common["bout"] = _f16(b_out.reshape(1, P))

    in_maps = []
    for c in range(NCORES):
        m = dict(common)
        m["zT"] = _f16(z[c * BS:(c + 1) * BS].T)
        in_maps.append(m)

    trace = os.environ.get("KERNEL_TRACE", "0") == "1"
    res = run_bass_kernel_spmd(nc, in_maps, core_ids=list(range(NCORES)),
                               trace=trace)
    last_exec_ns = res.exec_time_ns
    last_results = res
    # per-core results are (T, BS, P); un-transpose to (BS, T, P) on host
    out = np.stack([r["out"] for r in res.results], axis=0)   # [8, T, BS, P]
    return np.ascontiguousarray(out.transpose(0, 2, 1, 3)) \
             .reshape(B, T, P).astype(np.float32)


# revision 40
# speedup vs baseline: 1.0177x; 1.0177x over previous
"""Trainium2 Bass kernel for nn_Decoder: 2-layer GRU decoder + LayerNorm + ELU + vocab head.

Contract: kernel(**inputs) takes the FULL unsharded inputs and returns the FULL
(512, 64, 10000) float32 logits. Data-parallel: batch 512 -> 8 cores x 64.

Design (all-transposed, tanh-only, fp16):
- State kept TRANSPOSED the whole time: Shist [128 part = h-component-in-chunk,
  131 blocks, 2 h-chunks, 64 batch] fp16. Block q: q=0 init state; q=2s+1 =
  L0 state step s; q=2s+2 = L1 state (= output y1) step s. Slot s computes
  L0 step s and L1 step s-1 fused; their h' outputs land in ADJACENT blocks
  [2s, 2s+1] -> single fused DVE write. No PE transposes anywhere.
- Gate matmuls are weight-stationary ([128,128] fp16 lhsT chunks), moving
  operand = state block [128, 64]. Biases fold in as K=1 matmuls emitted
  FIRST in each PSUM accumulation group (they only read constants, so they
  run during the previous slot's ladder). r-gate matmuls are emitted before
  z (tanh-r gates the chain; z is only needed after tanh-n).
- sigmoid eliminated: r,z use t=tanh(x/2); r*hn = (1+tr)*hn' with the 0.5
  folded into the Whh n-rows on host; u=(1+tz)/2, v=(1-tz)/2 as off-chain
  DVE tensor_scalar ops. ACT therefore only needs Tanh/Exp/Square -> single
  table set (exp_and_others), so ELU = relu(y) + exp(y-relu(y)) - 1 with
  relu/exp on ACT, combines on DVE.
- LayerNorm in transposed layout: mean/E[h2] via ones-matmul partition
  reductions into PSUM, variance + rsqrt (bit-trick + 1 Newton iter) on
  GpSimd, broadcast back via K=1 ones matmuls into the SAME (time-shared)
  PSUM bank; apply+ELU fused over G=4 steps.
- Software pipeline per group g (slots 4g+4..4g+9): stats; apply+head-half;
  one head half-pair per slot thereafter. Head: yT (post-LN/ELU, fp16)
  stationary [128,128]; wout fp16 moving N=500; 1-bank PSUM tiles x5 bufs
  (avoids copy/matmul lockstep); PSUM->SBUF copies alternate DVE/ACT; one
  contiguous 2.56MB fp16 DMA per timestep-pair, alternating sync/gpsimd.
- Output DRAM laid out (T, BS, P) so pair DMAs are contiguous (scattered
  (b,t)-row writes measured 6x slower: 51 vs 297 GB/s); host un-transposes
  and casts fp16 -> f32.
"""
import os
import sys

for _p in ("/opt/trn_rl_repo", "/root/.axon_site/_ro/trn_rl_repo"):
    if os.path.isdir(_p) and _p not in sys.path:
        sys.path.append(_p)

import numpy as np
import concourse.bacc as bacc
import concourse.mybir as mybir
import concourse.tile as tile
from concourse.bass_utils import run_bass_kernel_spmd

F32 = mybir.dt.float32
F16 = mybir.dt.float16
I32 = mybir.dt.int32
AF = mybir.ActivationFunctionType
ALU = mybir.AluOpType

B, Z, H, T, P = 512, 64, 256, 64, 10000
NCORES = 8
BS = B // NCORES
LN_EPS = 1e-5
G = 4                  # LN/head group size (steps)
NCH = 500              # head N-chunk; 2 chunks (1024 cols w/ pad) per PSUM tile
RSQRT_NEWTON = 1

last_exec_ns = None
last_results = None


def _np(x):
    return np.ascontiguousarray(np.asarray(x, dtype=np.float32))


def _f16(x):
    return np.ascontiguousarray(np.asarray(x, dtype=np.float16))


def _build(flags):
    nc = bacc.Bacc("TRN2", target_bir_lowering=False)

    zT_d = nc.dram_tensor("zT", (Z, BS), F16, kind="ExternalInput")
    winitT_d = nc.dram_tensor("winitT", (Z, 2, 128), F16, kind="ExternalInput")
    whh0_d = nc.dram_tensor("whh0", (128, 2, 6, 128), F16, kind="ExternalInput")
    whh1_d = nc.dram_tensor("whh1", (128, 2, 6, 128), F16, kind="ExternalInput")
    wih1_d = nc.dram_tensor("wih1", (128, 2, 6, 128), F16, kind="ExternalInput")
    wout_d = nc.dram_tensor("wout", (128, 2, P), F16, kind="ExternalInput")
    c0rz_d = nc.dram_tensor("c0rz", (1, 2 * H), F16, kind="ExternalInput")
    c0n_d = nc.dram_tensor("c0n", (1, H), F16, kind="ExternalInput")
    opt = {}
    for name, shape in (
        ("binit", (1, H)), ("c1rz", (1, 2 * H)), ("bhh0n", (1, H)),
        ("bhh1n", (1, H)), ("bih1n", (1, H)), ("bout", (1, P)),
    ):
        if flags[name]:
            opt[name] = nc.dram_tensor(name, shape, F16, kind="ExternalInput")
    for name in ("lng", "lnb"):
        if flags[name]:
            opt[name] = nc.dram_tensor(name, (128, 2), F32, kind="ExternalInput")

    # (T, BS, P) layout so each timestep-pair staging DMA is one contiguous
    # 2.56MB block (scattered (b,t)-row writes measured 6x slower); the host
    # un-transposes to (BS, T, P).
    out_d = nc.dram_tensor("out", (T, BS, P), F16, kind="ExternalOutput")

    with tile.TileContext(nc) as tc:
        with (
            tc.tile_pool(name="const", bufs=1) as cp,
            tc.tile_pool(name="work", bufs=2) as wp,
            tc.tile_pool(name="psum", bufs=1, space="PSUM") as pp,
        ):
            # ---- constants / weights into SBUF -----------------------------
            zT = cp.tile([Z, BS], F16)
            winitT = cp.tile([Z, 2, 128], F16)
            whh0 = cp.tile([128, 2, 6, 128], F16)
            whh1 = cp.tile([128, 2, 6, 128], F16)
            wih1 = cp.tile([128, 2, 6, 128], F16)
            wout = cp.tile([128, 2, P], F16)
            c0rz = cp.tile([1, 2 * H], F16)
            c0n = cp.tile([1, H], F16)
            nc.sync.dma_start(out=zT, in_=zT_d[:])
            nc.sync.dma_start(out=winitT, in_=winitT_d[:])
            nc.scalar.dma_start(out=whh0, in_=whh0_d[:])
            nc.scalar.dma_start(out=whh1, in_=whh1_d[:])
            nc.scalar.dma_start(out=wih1, in_=wih1_d[:])
            nc.gpsimd.dma_start(out=wout, in_=wout_d[:])
            nc.sync.dma_start(out=c0rz, in_=c0rz_d[:])
            nc.sync.dma_start(out=c0n, in_=c0n_d[:])
            ot = {}
            for name, t_ in opt.items():
                sh = list(t_.shape)
                dt = F32 if name in ("lng", "lnb") else F16
                ot[name] = cp.tile(sh, dt)
                nc.sync.dma_start(out=ot[name], in_=t_[:])

            ones64 = cp.tile([1, BS], F16)
            nc.vector.memset(ones64, 1.0)
            ones1 = cp.tile([1, 128], F16)
            nc.vector.memset(ones1, 1.0)
            onesK = cp.tile([128, 1], F16)
            nc.vector.memset(onesK, 1.0 / H)

            # state history: block q=0 init; q=2s+1 L0 step s; q=2s+2 L1 step s
            Shist = cp.tile([128, 2 * T + 3, 2, BS], F16)

            # ---- init: h0 = elu(z @ W_init.T + b_init), write block 0 ------
            gi = pp.tile([128, 2, 8, BS], F32, tag="gates", bufs=1)
            for c in range(2):
                nc.tensor.matmul(gi[:, 0, c, :], winitT[:, c, :], zT,
                                 start=True, stop=not flags["binit"])
                if flags["binit"]:
                    nc.tensor.matmul(gi[:, 0, c, :],
                                     ot["binit"][:, c * 128:(c + 1) * 128],
                                     ones64, start=False, stop=True)
            h0pre = wp.tile([128, 2, BS], F16, tag="h0pre")
            nc.vector.tensor_scalar(out=h0pre, in0=gi[:, 0, 0:2, :], scalar1=0.0,
                                    scalar2=None, op0=ALU.min, op1=ALU.bypass)
            h0ex = wp.tile([128, 2, BS], F16, tag="h0ex")
            nc.scalar.activation(out=h0ex, in_=h0pre, func=AF.Exp)
            nc.vector.tensor_scalar(out=h0pre, in0=gi[:, 0, 0:2, :], scalar1=0.0,
                                    scalar2=None, op0=ALU.max, op1=ALU.bypass)
            nc.vector.scalar_tensor_tensor(out=Shist[:, 0, :, :], in0=h0ex,
                                           scalar=-1.0, in1=h0pre,
                                           op0=ALU.add, op1=ALU.add)

            c15 = cp.tile([1, G * BS], F32)
            nc.vector.memset(c15, 1.5)
            ceps = cp.tile([1, G * BS], F32)
            nc.vector.memset(ceps, LN_EPS)
            cm05 = cp.tile([1, G * BS], F32)
            nc.vector.memset(cm05, -0.5)

            # ---- helpers ---------------------------------------------------
            grp = {}
            grp_sb = {}

            def emit_stats(g):
                """Stage A: stats + rsqrt chain for L1 steps [G*g, G*g+G)."""
                # y1 step t lives at block 2t+2 -> stride-2 slice
                q0 = 2 * (G * g) + 2
                Hv = Shist[:, q0:q0 + 2 * G:2, :, :]        # [128, G(t), 2(c), 64]
                HvC = Hv.rearrange("p t c b -> p c t b")    # (c, t, b) view
                hvb = wp.tile([128, 2, G, BS], F16, tag="hvb")
                nc.vector.tensor_copy(out=hvb, in_=HvC)
                sq = wp.tile([128, 2, G, BS], F16, tag="sq")
                nc.scalar.activation(out=sq, in_=hvb, func=AF.Square)
                sb = pp.tile([128, 2, G * BS], F32, tag="sb", bufs=1)
                grp_sb[g] = sb
                st = sb[0:1, :, :]
                nc.tensor.matmul(st[:, 0, :], onesK, hvb[:, 0], start=True,
                                 stop=False)
                nc.tensor.matmul(st[:, 0, :], onesK, hvb[:, 1], start=False,
                                 stop=True)
                nc.tensor.matmul(st[:, 1, :], onesK, sq[:, 0], start=True,
                                 stop=False)
                nc.tensor.matmul(st[:, 1, :], onesK, sq[:, 1], start=False,
                                 stop=True)
                mv = wp.tile([1, 2, G * BS], F32, tag="mv")
                nc.vector.tensor_copy(out=mv, in_=st)
                # ve = var + eps; vh = -0.5*ve  (gpsimd; mv[0]=mu, mv[1]=E[h^2])
                ve = wp.tile([1, G * BS], F32, tag="ve")
                vh = wp.tile([1, G * BS], F32, tag="vh")
                nc.gpsimd.tensor_tensor(out=ve, in0=mv[:, 0, :], in1=mv[:, 0, :],
                                        op=ALU.mult)
                nc.gpsimd.tensor_tensor(out=ve, in0=mv[:, 1, :], in1=ve,
                                        op=ALU.subtract)
                nc.gpsimd.tensor_tensor(out=ve, in0=ve, in1=ceps, op=ALU.add)
                nc.gpsimd.tensor_tensor(out=vh, in0=ve, in1=cm05, op=ALU.mult)
                # rsqrt(ve): bit trick + Newton (tensor-tensor only on gpsimd)
                yi = wp.tile([1, G * BS], I32, tag="yi")
                nc.vector.tensor_scalar(out=yi, in0=ve.bitcast(I32), scalar1=1,
                                        scalar2=None, op0=ALU.logical_shift_right,
                                        op1=ALU.bypass)
                nc.vector.tensor_scalar(out=yi, in0=yi, scalar1=-1,
                                        scalar2=0x5F3759DF, op0=ALU.mult,
                                        op1=ALU.add)
                rs = yi.bitcast(F32)
                tn = wp.tile([1, G * BS], F32, tag="tn")
                for _ in range(RSQRT_NEWTON):
                    nc.gpsimd.tensor_tensor(out=tn, in0=rs, in1=rs, op=ALU.mult)
                    nc.gpsimd.tensor_tensor(out=tn, in0=tn, in1=vh, op=ALU.mult)
                    nc.gpsimd.tensor_tensor(out=tn, in0=tn, in1=c15, op=ALU.add)
                    nc.gpsimd.tensor_tensor(out=rs, in0=rs, in1=tn, op=ALU.mult)
                # fp16 (mu, rs) for broadcast matmuls
                m16 = wp.tile([1, 2, G * BS], F16, tag="m16")
                nc.vector.tensor_copy(out=m16[:, 0, :], in_=mv[:, 0, :])
                nc.vector.tensor_copy(out=m16[:, 1, :], in_=rs)
                grp[g] = {"hvb": hvb, "m16": m16}

            def emit_apply(g):
                """Stage B: broadcast + LN apply + ELU -> yb (fp16, (c,t,b))."""
                d = grp[g]
                hvb, m16 = d["hvb"], d["m16"]
                bc = grp_sb.pop(g)
                nc.tensor.matmul(bc[:, 0, :], ones1, m16[:, 0, :],
                                 start=True, stop=True)
                nc.tensor.matmul(bc[:, 1, :], ones1, m16[:, 1, :],
                                 start=True, stop=True)
                # apply LN: y = (h - mu) * rs  (+ lng/lnb if present)
                bmu = bc[:, 0, :].rearrange("p (t b) -> p t b", t=G) \
                    .unsqueeze(1).broadcast_to([128, 2, G, BS])
                brs = bc[:, 1, :].rearrange("p (t b) -> p t b", t=G) \
                    .unsqueeze(1).broadcast_to([128, 2, G, BS])
                yb = wp.tile([128, 2, G, BS], F16, tag="yb")
                nc.vector.tensor_tensor(out=yb, in0=hvb, in1=bmu, op=ALU.subtract)
                nc.vector.tensor_tensor(out=yb, in0=yb, in1=brs, op=ALU.mult)
                d["yb"] = yb
                if flags["lng"]:
                    for c in range(2):
                        nc.vector.tensor_scalar(out=yb[:, c], in0=yb[:, c],
                                                scalar1=ot["lng"][:, c:c + 1],
                                                scalar2=None, op0=ALU.mult,
                                                op1=ALU.bypass)
                if flags["lnb"]:
                    for c in range(2):
                        nc.vector.tensor_scalar(out=yb[:, c], in0=yb[:, c],
                                                scalar1=ot["lnb"][:, c:c + 1],
                                                scalar2=None, op0=ALU.add,
                                                op1=ALU.bypass)
                # ELU: yp = relu(y); y = exp(y - yp) - 1 + yp
                yp = wp.tile([128, 2, G, BS], F16, tag="yp")
                nc.scalar.activation(out=yp, in_=yb, func=AF.Relu)
                mn = wp.tile([128, 2, G, BS], F16, tag="mn")
                nc.vector.scalar_tensor_tensor(out=mn, in0=yp, scalar=-1.0,
                                               in1=yb, op0=ALU.mult, op1=ALU.add)
                ex = wp.tile([128, 2, G, BS], F16, tag="ex")
                nc.scalar.activation(out=ex, in_=mn, func=AF.Exp)
                nc.vector.scalar_tensor_tensor(out=yb, in0=ex, scalar=-1.0,
                                               in1=yp, op0=ALU.add, op1=ALU.add)

            def emit_half(g, j, h):
                """Half of the head work (5 of 10 q-groups) for pair j of
                group g; the DMA is issued with the second half."""
                yb = grp[g]["yb"]
                t0 = G * g + 2 * j
                if h == 0:
                    stg = wp.tile([128, P], F16, tag="stg", bufs=3)
                    grp[g]["stg%d" % j] = stg
                else:
                    stg = grp[g]["stg%d" % j]
                yT0 = yb[:, 0, 2 * j:2 * j + 2, :]
                yT1 = yb[:, 1, 2 * j:2 * j + 2, :]
                nq = P // NCH
                for n in range(h * nq // 2, (h + 1) * nq // 2):
                    hp = pp.tile([128, 512], F32, tag="hp", bufs=5)
                    nc.tensor.matmul(hp[:, 0:NCH], yT0,
                                     wout[:, 0, n * NCH:(n + 1) * NCH],
                                     start=True, stop=False)
                    nc.tensor.matmul(hp[:, 0:NCH], yT1,
                                     wout[:, 1, n * NCH:(n + 1) * NCH],
                                     start=False, stop=not flags["bout"])
                    if flags["bout"]:
                        nc.tensor.matmul(hp[:, 0:NCH], ones1,
                                         ot["bout"][:, n * NCH:(n + 1) * NCH],
                                         start=False, stop=True)
                    dst = stg[:, n * NCH:(n + 1) * NCH]
                    if n % 2 == 0:
                        nc.vector.tensor_copy(out=dst, in_=hp[:, 0:NCH])
                    else:
                        nc.scalar.copy(out=dst, in_=hp[:, 0:NCH])
                if h == 1:
                    eng = (nc.sync, nc.gpsimd)[(2 * g + j) % 2]
                    eng.dma_start(out=out_d[t0:t0 + 2], in_=stg)

            # gate tile slice layout: [128, l(2), kind(8), 64]
            # l: 0=L1, 1=L0.  kind: 0,1=r(c0,c1) 2,3=z 4,5=hn' 6,7=xn
            # ---- main loop -------------------------------------------------
            for s in range(T + 1):
                L0 = s < T
                L1 = s >= 1
                lo = 0 if L1 else 1      # active l-slice range [lo:hi]
                hi = 2 if L0 else 1
                nl = hi - lo

                hp_ctx = tc.high_priority()
                hp_ctx.__enter__()
                gt = pp.tile([128, 2, 8, BS], F32, tag="gates", bufs=1)

                # emission order matters: tanh-r gates only on the r-chunk
                # matmuls, so emit r first, then hn/xn (needed next by th/ta),
                # and z last (needed only after tanh-n).
                rhs_h1 = Shist[:, max(2 * s - 2, 0), :, :]       # y1_{s-2}
                rhs_y0 = Shist[:, max(2 * s - 1, 0), :, :]       # y0_{s-1}
                rhs_h0 = Shist[:, max(2 * s - 1, 0), :, :]       # h0_{s-1}

                def l1_rz(gc):
                    # bias first: it only reads constants, so it can execute
                    # during the previous slot's ladder
                    if flags["c1rz"]:
                        nc.tensor.matmul(gt[:, 0, gc, :],
                                         ot["c1rz"][:, gc * 128:(gc + 1) * 128],
                                         ones64, start=True, stop=False)
                    nc.tensor.matmul(gt[:, 0, gc, :], whh1[:, 0, gc, :],
                                     rhs_h1[:, 0, :], start=not flags["c1rz"],
                                     stop=False)
                    nc.tensor.matmul(gt[:, 0, gc, :], whh1[:, 1, gc, :],
                                     rhs_h1[:, 1, :], start=False, stop=False)
                    nc.tensor.matmul(gt[:, 0, gc, :], wih1[:, 0, gc, :],
                                     rhs_y0[:, 0, :], start=False, stop=False)
                    nc.tensor.matmul(gt[:, 0, gc, :], wih1[:, 1, gc, :],
                                     rhs_y0[:, 1, :], start=False, stop=True)

                def l0_rz(gc):
                    nc.tensor.matmul(gt[:, 1, gc, :],
                                     c0rz[:, gc * 128:(gc + 1) * 128],
                                     ones64, start=True, stop=False)
                    nc.tensor.matmul(gt[:, 1, gc, :], whh0[:, 0, gc, :],
                                     rhs_h0[:, 0, :], start=False, stop=False)
                    nc.tensor.matmul(gt[:, 1, gc, :], whh0[:, 1, gc, :],
                                     rhs_h0[:, 1, :], start=False, stop=True)

                for gc in (0, 1):                                # r chunks
                    if L1:
                        l1_rz(gc)
                    if L0:
                        l0_rz(gc)
                if L1:
                    for ci in range(2):                          # hn', xn
                        nc.tensor.matmul(gt[:, 0, 4 + ci, :], whh1[:, 0, 4 + ci, :],
                                         rhs_h1[:, 0, :], start=True, stop=False)
                        nc.tensor.matmul(gt[:, 0, 4 + ci, :], whh1[:, 1, 4 + ci, :],
                                         rhs_h1[:, 1, :], start=False,
                                         stop=not flags["bhh1n"])
                        if flags["bhh1n"]:
                            nc.tensor.matmul(gt[:, 0, 4 + ci, :],
                                             ot["bhh1n"][:, ci * 128:(ci + 1) * 128],
                                             ones64, start=False, stop=True)
                        nc.tensor.matmul(gt[:, 0, 6 + ci, :], wih1[:, 0, 4 + ci, :],
                                         rhs_y0[:, 0, :], start=True, stop=False)
                        nc.tensor.matmul(gt[:, 0, 6 + ci, :], wih1[:, 1, 4 + ci, :],
                                         rhs_y0[:, 1, :], start=False,
                                         stop=not flags["bih1n"])
                        if flags["bih1n"]:
                            nc.tensor.matmul(gt[:, 0, 6 + ci, :],
                                             ot["bih1n"][:, ci * 128:(ci + 1) * 128],
                                             ones64, start=False, stop=True)
                if L0:
                    for ci in range(2):
                        nc.tensor.matmul(gt[:, 1, 4 + ci, :], whh0[:, 0, 4 + ci, :],
                                         rhs_h0[:, 0, :], start=True, stop=False)
                        nc.tensor.matmul(gt[:, 1, 4 + ci, :], whh0[:, 1, 4 + ci, :],
                                         rhs_h0[:, 1, :], start=False,
                                         stop=not flags["bhh0n"])
                        if flags["bhh0n"]:
                            nc.tensor.matmul(gt[:, 1, 4 + ci, :],
                                             ot["bhh0n"][:, ci * 128:(ci + 1) * 128],
                                             ones64, start=False, stop=True)
                        nc.tensor.matmul(gt[:, 1, 6 + ci, :],
                                         c0n[:, ci * 128:(ci + 1) * 128],
                                         ones64, start=True, stop=True)
                for gc in (2, 3):                                # z chunks (late)
                    if L1:
                        l1_rz(gc)
                    if L0:
                        l0_rz(gc)

                # nonlinearity + blend (fused over active layers)
                # u = (1+tz)/2, v = (1-tz)/2; h' = u*h_prev + v*n
                tr = wp.tile([128, 2, 2, BS], F16, tag="tr")
                tz = wp.tile([128, 2, 2, BS], F16, tag="tz")
                th = wp.tile([128, 2, 2, BS], F16, tag="th")
                ta = wp.tile([128, 2, 2, BS], F16, tag="ta")
                nn = wp.tile([128, 2, 2, BS], F16, tag="nn")
                uu = wp.tile([128, 2, 2, BS], F16, tag="uu")
                vv = wp.tile([128, 2, 2, BS], F16, tag="vv")
                uh = wp.tile([128, 2, 2, BS], F16, tag="uh")
                vn = wp.tile([128, 2, 2, BS], F16, tag="vn")

                A = slice(lo, hi)
                nc.scalar.activation(out=tr[:, A], in_=gt[:, A, 0:2, :],
                                     func=AF.Tanh, scale=0.5)
                nc.vector.scalar_tensor_tensor(out=th[:, A], in0=tr[:, A],
                                               scalar=1.0, in1=gt[:, A, 4:6, :],
                                               op0=ALU.add, op1=ALU.mult)
                nc.vector.tensor_tensor(out=ta[:, A], in0=th[:, A],
                                        in1=gt[:, A, 6:8, :], op=ALU.add)
                nc.scalar.activation(out=tz[:, A], in_=gt[:, A, 2:4, :],
                                     func=AF.Tanh, scale=0.5)
                nc.scalar.activation(out=nn[:, A], in_=ta[:, A], func=AF.Tanh)
                # h_prev blocks: [2s-2]=y1_{s-2}, [2s-1]=h0_{s-1}
                if s == 0:
                    hprev = Shist[:, 0:1, :, :]
                else:
                    base = 2 * s - 2 + lo
                    hprev = Shist[:, base:base + nl, :, :]
                # u/v/uh run on DVE while ACT computes tanh-n
                nc.vector.tensor_scalar(out=uu[:, A], in0=tz[:, A], scalar1=0.5,
                                        scalar2=0.5, op0=ALU.mult, op1=ALU.add)
                nc.vector.tensor_scalar(out=vv[:, A], in0=tz[:, A], scalar1=-0.5,
                                        scalar2=0.5, op0=ALU.mult, op1=ALU.add)
                nc.vector.tensor_tensor(out=uh[:, A], in0=uu[:, A],
                                        in1=hprev, op=ALU.mult)
                nc.vector.tensor_tensor(out=vn[:, A], in0=vv[:, A],
                                        in1=nn[:, A], op=ALU.mult)
                nc.vector.tensor_tensor(
                    out=Shist[:, 2 * s + lo:2 * s + hi, :, :],
                    in0=uh[:, A], in1=vn[:, A], op=ALU.add)
                hp_ctx.__exit__(None, None, None)

                # software-pipelined phase B (one head half-pair per slot):
                #   4g+4: stats(g);  4g+6: apply(g) + pair0.h0;  4g+7: pair0.h1
                #   4g+8: pair1.h0;  4g+9: pair1.h1
                r = s % G
                if r == 0 and s >= G:
                    emit_stats(s // G - 1)
                if r == 2 and s >= 6:
                    emit_apply((s - 6) // G)
                    emit_half((s - 6) // G, 0, 0)
                if r == 3 and s >= 7:
                    emit_half((s - 7) // G, 0, 1)
                if r == 0 and s >= 8:
                    emit_half((s - 8) // G, 1, 0)
                if r == 1 and s >= 9:
                    emit_half((s - 9) // G, 1, 1)

            # tail: finish the pipeline for the last groups
            emit_half(T // G - 2, 1, 1)
            emit_apply(T // G - 1)
            emit_half(T // G - 1, 0, 0)
            emit_half(T // G - 1, 0, 1)
            emit_half(T // G - 1, 1, 0)
            emit_half(T // G - 1, 1, 1)

    nc.compile()
    return nc


_cache = {}


def _get_program(flags):
    key = tuple(sorted(flags.items()))
    if key not in _cache:
        _cache[key] = _build(flags)
    return _cache[key]


def kernel(z, W_init, b_init, embedding, W_ih0, W_hh0, b_ih0, b_hh0,
           W_ih1, W_hh1, b_ih1, b_hh1, ln_g, ln_b, W_out, b_out):
    global last_exec_ns, last_results
    z = _np(z); W_init = _np(W_init); b_init = _np(b_init)
    embedding = _np(embedding)
    W_ih0 = _np(W_ih0); W_hh0 = _np(W_hh0); b_ih0 = _np(b_ih0); b_hh0 = _np(b_hh0)
    W_ih1 = _np(W_ih1); W_hh1 = _np(W_hh1); b_ih1 = _np(b_ih1); b_hh1 = _np(b_hh1)
    ln_g = _np(ln_g); ln_b = _np(ln_b); W_out = _np(W_out); b_out = _np(b_out)

    # layer-0 input gates are constant across (b, t): fold embedding @ W_ih0.T
    gx0 = (embedding @ W_ih0.T + b_ih0).reshape(1, 3 * H)
    c0rz = gx0[:, 0:2 * H] + b_hh0[None, 0:2 * H]
    c0n = gx0[:, 2 * H:]
    c1rz = (b_ih1 + b_hh1)[None, 0:2 * H]

    flags = {
        "binit": bool(np.any(b_init != 0)),
        "c1rz": bool(np.any(c1rz != 0)),
        "bhh0n": bool(np.any(b_hh0[2 * H:] != 0)),
        "bhh1n": bool(np.any(b_hh1[2 * H:] != 0)),
        "bih1n": bool(np.any(b_ih1[2 * H:] != 0)),
        "lng": bool(np.any(ln_g != 1.0)),
        "lnb": bool(np.any(ln_b != 0)),
        "bout": bool(np.any(b_out != 0)),
    }
    nc = _get_program(flags)

    # weight prep: transposed chunked layouts; 0.5 folded into Whh n-rows
    Wt0 = W_hh0.T.copy()
    Wt0[:, 2 * H:] *= 0.5
    Wt1 = W_hh1.T.copy()
    Wt1[:, 2 * H:] *= 0.5
    common = {
        "winitT": _f16(W_init.T.reshape(Z, 2, 128)),
        "whh0": _f16(Wt0.reshape(2, 128, 6, 128).transpose(1, 0, 2, 3)),
        "whh1": _f16(Wt1.reshape(2, 128, 6, 128).transpose(1, 0, 2, 3)),
        "wih1": _f16(W_ih1.T.reshape(2, 128, 6, 128).transpose(1, 0, 2, 3)),
        "wout": _f16(W_out.T.reshape(2, 128, P).transpose(1, 0, 2)),
        "c0rz": _f16(c0rz),
        "c0n": _f16(c0n),
    }
    if flags["binit"]:
        common["binit"] = _f16(b_init.reshape(1, H))
    if flags["c1rz"]:
        common["c1rz"] = _f16(c1rz)
    if flags["bhh0n"]:
        common["bhh0n"] = _f16(0.5 * b_hh0[None, 2 * H:])
    if flags["bhh1n"]:
        common["bhh1n"] = _f16(0.5 * b_hh1[None, 2 * H:])
    if flags["bih1n"]:
        common["bih1n"] = _f16(b_ih1[None, 2 * H:])
    if flags["lng"]:
        common["lng"] = np.ascontiguousarray(ln_g.reshape(2, 128).T)
    if flags["lnb"]:
        common["lnb"] = np.ascontiguousarray(ln_b.reshape(2, 128).T)
    if flags["bout"]:
        common["bout"] = _f16(b_out.reshape(1, P))

    in_maps = []
    for c in range(NCORES):
        m = dict(common)
        m["zT"] = _f16(z[c * BS:(c + 1) * BS].T)
        in_maps.append(m)

    trace = os.environ.get("KERNEL_TRACE", "0") == "1"
    res = run_bass_kernel_spmd(nc, in_maps, core_ids=list(range(NCORES)),
                               trace=trace)
    last_exec_ns = res.exec_time_ns
    last_results = res
    # per-core results are (T, BS, P); un-transpose to (BS, T, P) on host
    out = np.stack([r["out"] for r in res.results], axis=0)   # [8, T, BS, P]
    return np.ascontiguousarray(out.transpose(0, 2, 1, 3)) \
             .reshape(B, T, P).astype(np.float32)
